# revision 59
# baseline (speedup 1.0000x reference)
"""ChildSum TreeLSTM (complete binary tree, depth 17) on 8 trn2 NeuronCores.

Strategy
--------
The tree (262143 nodes, level-major) is split at global level 3 into 8
subtrees of 32767 nodes; core k reduces subtree k bottom-up over the levels
that sustain a fully pipelined steady state (local leaf level ll=14 down to
ll=11, i.e. every level with >= 2 full 1024-wide chunks) entirely on-chip.
The serial remainder (global levels 13..0, 16383 nodes, 6% of the tree and
<2% of FLOPs, where per-core level width collapses below the pipeline
granularity) is merged on the host in fp32, per the "all-gather child h/c
at the subtree merge levels near the root" sharding hint.

Layout: everything on-device is feature-major ([H=128 partitions, nodes on
the free axis]) so that
  pre_g^T = W_g^T @ x^T + U_g^T @ h^T
is two PSUM-accumulated matmuls with the stored (in,out) weights as lhsT,
and the per-gate bias rides the ScalarE activation's per-partition bias.

Each level's nodes are stored in *bit-reversed* order: the children of the
parent at position p sit at position p of the first and second half of the
child level. Every on-chip access (child-sum, forget gates, f*c products)
is then unit-stride. The host builds the per-core x^T with this permutation
baked in (bf16, which also halves HBM traffic), with the gate weights packed
into the leading 1024 columns (xpack) so one DMA lands W/U plus the first
leaf chunk at kernel start.

The Act (scalar) engine is the bottleneck (6 nonlinearity elems per internal
node, 4 per leaf, at 1 elem/cycle/partition @1.2GHz): the schedule keeps it
>90% busy through every on-device level. h/c level buffers ping-pong by
level parity to halve SBUF footprint; gpool bufs=3 deepens cross-chunk
pipelining. DVE-polynomial tanh offload and PSUM-bank reuse tricks were
benched on HW and lose (see memory notes): scalar_tensor_tensor costs ~2x
its model and a second PSUM accumulation group in a reused bank corrupts.
"""

import numpy as np
import ml_dtypes

import concourse.bass as bass
import concourse.tile as tile
from concourse import bacc
from concourse import mybir
from concourse.bass_utils import run_bass_kernel_spmd

DEPTH = 17
H = 128
SPLIT = 3                    # subtree roots at global level 3 -> 8 subtrees
NCORES = 8
N_LL = DEPTH - SPLIT         # local leaf level (14); device runs ll=N_LL..1
NSUB = 2 ** (N_LL + 1) - 1   # nodes per subtree = x^T columns per core
CHUNK = 1024                 # free-dim chunk (two fp32 PSUM banks per gate)
# best-known configuration (sim + HW A/B): wide chunks, single-buffered
# 2-bank i/o/u PSUM tiles, 2-bank f PSUM, weight-grouped matmuls
# tail_fmerge is OFF: two start=True groups in one PSUM bank corrupt the
# first group's accumulation when P < 512 (verified on HW small-scale).
BUILD_OPTS = {"wide": True, "f_width": 1024, "pf_bufs": 1,
              "iou_bufs": 1, "gpool_bufs": 3, "stop_ll": 11,
              "io2": False, "x0pf": True, "parity": True, "t01": False,
              "wfirst": False, "tail_fast": True, "f1pi": False, "xpack": True,
              "lsplit": 0, "pool_h": False, "u2nd": False}

BF16 = mybir.dt.bfloat16
F32 = mybir.dt.float32

GATES = ("i", "f", "o", "u")

TRACE = False   # set by test.py to capture an NTFF profile
LAST = None     # BassKernelResults of the most recent run
SIG = mybir.ActivationFunctionType.Sigmoid
TANH = mybir.ActivationFunctionType.Tanh


def _emit_body_wide(nc, tc, xt, W, U, b, n_ll, chunk,
                    xpool, xtailpool, gpool, ppool, pfpool, hcpool, out_hc, opts={}):
    """chunk=1024 variant: i/o/u gates span two PSUM banks and get ONE
    activation op each; f-gates run at 512 width (PSUM budget); matmuls are
    emitted grouped by stationary weight so LDWEIGHTS amortizes over pairs."""
    stop_ll = opts.get("stop_ll", 1)
    MMW = min(512, chunk)
    assert chunk % MMW == 0
    io2 = opts.get("io2")
    if io2:
        bT, ones = opts["bT"], opts["ones"]

    xoff = opts.get("xoff", 0)   # xpack: weights occupy the first xt columns

    def lvl_off(ll):
        if xoff:   # packed layout: [wu | leaf | ll-1 | ... | 0] (read order)
            return xoff + (2 ** (n_ll + 1) - 2 ** (ll + 1))
        return 2 ** ll - 1

    def x_chunk(ll, a, P):
        if ll == n_ll and a == 0 and opts.get("x0") is not None:
            return opts["x0"]       # prefetched with the weights
        off = lvl_off(ll)
        pool = xpool if 2 ** ll > chunk else xtailpool
        xt_sb = pool.tile([H, P], BF16, tag="x")
        nc.sync.dma_start(out=xt_sb, in_=xt[:, off + a : off + a + P])
        return xt_sb

    def gate_mms(ps, P, srcs):
        # srcs: list of (lhsT, rhs_tile_slicer); emit grouped by weight
        for si, (lhsT, rhs) in enumerate(srcs):
            first = si == 0
            last = si == len(srcs) - 1
            for s in range(0, P, MMW):
                w = min(MMW, P - s)
                nc.tensor.matmul(ps[:, s : s + w], lhsT, rhs[:, s : s + w],
                                 start=first, stop=last)

    def iou_gates(P, xs, ht):
        """Returns (gi, go, gu). io2: one 2P-wide sigmoid for i|o with the
        per-gate bias accumulated in PSUM via a ones-row matmul."""
        if io2:
            pio = ppool.tile([H, 2 * P], F32, tag="pio", name="pio")
            for j, g in enumerate(("i", "o")):
                srcs = ([] if ht is None else [(U[g], ht)]) + \
                       [(W[g], xs), (bT[:, j, :], ones[:, 0:P])]
                gate_mms(pio[:, j * P : (j + 1) * P], P, srcs)
            gio = gpool.tile([H, 2 * P], BF16, tag="gio", name="gio")
            nc.scalar.activation(gio, pio, SIG)
            gi, go = gio[:, 0:P], gio[:, P : 2 * P]
        else:
            gts = {}
            order = ("i", "u", "o") if opts.get("u2nd") else ("i", "o", "u")
            for g in order:
                tag = "pu" if g == "u" else f"p{g}"
                ps = ppool.tile([H, P], F32, tag=tag, name=tag)
                gate_mms(ps, P, ([] if ht is None else [(U[g], ht)]) + [(W[g], xs)])
                gts[g] = gpool.tile([H, P], BF16, tag=f"g{g}", name=f"g{g}")
                nc.scalar.activation(gts[g], ps, TANH if g == "u" else SIG,
                                     bias=b[g])
            gi, go, gu = gts["i"], gts["o"], gts["u"]
        return gi, go, gu

    # ---- leaves ----
    # lsplit: for the first `lsplit` leaf chunks, tanh(c) runs on the DVE as
    # a degree-5 odd minimax polynomial (|c|<1 at leaves since c = i*u), and
    # h = o*tanh(c) runs on the otherwise-idle Pool engine. This moves work
    # off the saturated Act engine; max poly+bf16 err ~5e-3 per leaf h.
    K0, K1, K2 = 0.99716336, -0.30798793, 0.0728065
    lsplit = opts.get("lsplit", 0)
    pool_h = opts.get("pool_h", False)
    k1t = None
    if lsplit:
        k1t = opts["k1t"]
    nl = 2 ** n_ll
    tagm = 2 if opts.get("parity", True) else 10000
    h_prev = hcpool.tile([H, nl], BF16, tag=f"h{n_ll % tagm}")
    c_prev = hcpool.tile([H, nl], BF16, tag=f"c{n_ll % tagm}")
    nchunks = max(1, nl // chunk)
    ht_pre = None
    for ci, a in enumerate(range(0, nl, chunk)):
        P = min(chunk, nl - a)
        xs = x_chunk(n_ll, a, P)
        gi, go, gu = iou_gates(P, xs, None)
        cs = c_prev[:, a : a + P]
        nc.vector.tensor_mul(cs, gi, gu)
        # spread the DVE-offloaded chunks evenly among the Act ones so both
        # engines stay busy (Bresenham selection of `lsplit` of nchunks)
        off = ((ci + 1) * lsplit) // nchunks > (ci * lsplit) // nchunks
        if off:
            t2 = gpool.tile([H, P], BF16, tag="t2", name="t2")
            nc.vector.tensor_mul(t2, cs, cs)
            pv = gpool.tile([H, P], BF16, tag="pv", name="pv")
            if opts.get("ttpoly"):
                k0t, k2t = opts["k0t"], opts["k2t"]
                nc.vector.tensor_mul(pv, t2, k2t[:, 0:P])
                nc.vector.tensor_add(pv, pv, k1t[:, 0:P])
                pw = gpool.tile([H, P], BF16, tag="pw", name="pw")
                nc.vector.tensor_mul(pw, pv, t2)
                nc.vector.tensor_add(pw, pw, k0t[:, 0:P])
                tct = gpool.tile([H, P], BF16, tag="tanhc", name="tanhc")
                nc.vector.tensor_mul(tct, pw, cs)
            else:
                nc.vector.scalar_tensor_tensor(pv, in0=t2, scalar=float(K2),
                                               in1=k1t[:, 0:P],
                                               op0=mybir.AluOpType.mult,
                                               op1=mybir.AluOpType.add)
                pw = gpool.tile([H, P], BF16, tag="pw", name="pw")
                nc.vector.tensor_mul(pw, pv, t2)
                tct = gpool.tile([H, P], BF16, tag="tanhc", name="tanhc")
                nc.vector.scalar_tensor_tensor(tct, in0=pw, scalar=float(K0),
                                               in1=cs,
                                               op0=mybir.AluOpType.add,
                                               op1=mybir.AluOpType.mult)
        else:
            tct = gpool.tile([H, P], BF16, tag="tanhc", name="tanhc")
            nc.scalar.activation(tct, cs, TANH)
        if pool_h or off:
            nc.gpsimd.tensor_mul(h_prev[:, a : a + P], go, tct)
        else:
            nc.vector.tensor_mul(h_prev[:, a : a + P], go, tct)
        if (opts.get("ht0", True) and n_ll - 1 >= stop_ll
                and nl >= 2 * chunk and a == nl // 2):
            # hoist the next level's first-chunk child-sum: its inputs
            # (h[0:chunk] and h[nl/2:nl/2+chunk]) are complete now, so the
            # in-order DVE computes it mid-level instead of after the last
            # tanh/h chain — removes the Act stall at the level boundary
            ht_pre = gpool.tile([H, chunk], BF16, tag="htp", name="htp")
            nc.vector.tensor_add(ht_pre, h_prev[:, 0:chunk],
                                 h_prev[:, nl // 2 : nl // 2 + chunk])

    # ---- internal levels ----
    for ll in range(n_ll - 1, stop_ll - 1, -1):
        nl = 2 ** ll
        h_cur = hcpool.tile([H, nl], BF16, tag=f"h{ll % tagm}")
        c_cur = hcpool.tile([H, nl], BF16, tag=f"c{ll % tagm}")
        for a in range(0, nl, chunk):
            P = min(chunk, nl - a)
            xs = x_chunk(ll, a, P)
            h0 = h_prev[:, a : a + P]
            h1 = h_prev[:, nl + a : nl + a + P]
            c0 = c_prev[:, a : a + P]
            c1 = c_prev[:, nl + a : nl + a + P]
            if a == 0 and ht_pre is not None:
                ht, ht_hoisted = ht_pre, True
                ht_pre = None
            else:
                ht = gpool.tile([H, P], BF16, tag="ht")
                ht_hoisted = False
            single = P == nl and opts.get("tail_fast", True)
            wfirst = single or (a == 0 and opts.get("wfirst", True))
            if wfirst:
                # level-boundary chunk: emit every W-pass (start=True) before
                # the h-dependent work so PE fills the level-boundary drain,
                # then the U-passes (stop=True) + activations as h arrives.
                ps = {}
                for g in ("i", "o", "u"):
                    ps[g] = ppool.tile([H, P], F32, tag=f"p{g}" if g != "u" else "pu",
                                       name=f"p{g}")
                    gate_mms(ps[g], P, [(W[g], xs)])
                psf0 = pfpool.tile([H, opts.get("f_width", MMW)], F32,
                                   tag="pf", name="psf0")
                for q in range(0, P, MMW):
                    qw = min(MMW, P - q)
                    nc.tensor.matmul(psf0[:, q : q + qw], W["f"],
                                     xs[:, q : q + qw], start=True, stop=False)
                if not ht_hoisted:
                    nc.vector.tensor_add(ht, h0, h1)
                gts = {}
                for g, fn in (("i", SIG), ("o", SIG), ("u", TANH)):
                    for q in range(0, P, MMW):
                        qw = min(MMW, P - q)
                        nc.tensor.matmul(ps[g][:, q : q + qw], U[g],
                                         ht[:, q : q + qw], start=False, stop=True)
                    gts[g] = gpool.tile([H, P], BF16, tag=f"g{g}", name=f"g{g}")
                    nc.scalar.activation(gts[g], ps[g], fn, bias=b[g])
                gi, go, gu = gts["i"], gts["o"], gts["u"]
                gf = gpool.tile([H, 2 * P], BF16, tag="gf")
                for q in range(0, P, MMW):
                    qw = min(MMW, P - q)
                    nc.tensor.matmul(psf0[:, q : q + qw], U["f"],
                                     h0[:, q : q + qw], start=False, stop=True)
                nc.scalar.activation(gf[:, 0:P], psf0[:, 0:P], SIG, bias=b["f"])
                if single and opts.get("f1pi", True):
                    # f1 reuses the i-gate's PSUM bank (freed once σi read it)
                    psf1 = ppool.tile([H, P], F32, tag="pi", name="psf1")
                else:
                    psf1 = pfpool.tile([H, opts.get("f_width", MMW)], F32,
                                       tag="pf", name="psf1")
                gate_mms(psf1[:, 0:P], P, [(W["f"], xs), (U["f"], h1)])
                nc.scalar.activation(gf[:, P : 2 * P], psf1[:, 0:P], SIG,
                                     bias=b["f"])
            else:
                if not ht_hoisted:
                    nc.vector.tensor_add(ht, h0, h1)
                gi, go, gu = iou_gates(P, xs, ht)
                gf = gpool.tile([H, 2 * P], BF16, tag="gf")
                f_w = opts.get("f_width", MMW)
                for j, hj in ((0, h0), (1, h1)):
                    for s in range(0, P, f_w):
                        w = min(f_w, P - s)
                        psf = pfpool.tile([H, f_w], F32, tag="pf", name="psf")
                        for q in range(0, w, MMW):
                            qw = min(MMW, w - q)
                            nc.tensor.matmul(psf[:, q : q + qw], U["f"],
                                             hj[:, s + q : s + q + qw],
                                             start=True, stop=False)
                        for q in range(0, w, MMW):
                            qw = min(MMW, w - q)
                            nc.tensor.matmul(psf[:, q : q + qw], W["f"],
                                             xs[:, s + q : s + q + qw],
                                             start=False, stop=True)
                        nc.scalar.activation(gf[:, j * P + s : j * P + s + w],
                                             psf[:, 0:w], SIG, bias=b["f"])
            iu = gpool.tile([H, P], BF16, tag="iu")
            nc.vector.tensor_mul(iu, gi, gu)
            if P == nl and opts.get("t01", True):
                # single-chunk level: c0|c1 contiguous in c_prev
                t01 = gpool.tile([H, 2 * P], BF16, tag="t0", name="t01")
                nc.vector.tensor_mul(t01, gf, c_prev[:, 0 : 2 * P])
                ts = gpool.tile([H, P], BF16, tag="ts")
                nc.vector.tensor_add(ts, t01[:, 0:P], t01[:, P : 2 * P])
            else:
                t0 = gpool.tile([H, P], BF16, tag="t0")
                nc.vector.tensor_mul(t0, gf[:, 0:P], c0)
                t1 = gpool.tile([H, P], BF16, tag="t1")
                nc.vector.tensor_mul(t1, gf[:, P : 2 * P], c1)
                ts = gpool.tile([H, P], BF16, tag="ts")
                nc.vector.tensor_add(ts, t0, t1)
            nc.vector.tensor_add(c_cur[:, a : a + P], iu, ts)
            tct = gpool.tile([H, P], BF16, tag="tanhc", name="tanhc")
            nc.scalar.activation(tct, c_cur[:, a : a + P], TANH)
            nc.vector.tensor_mul(h_cur[:, a : a + P], go, tct)
            if (opts.get("ht0", True) and ll - 1 >= stop_ll
                    and nl >= 2 * chunk and a == nl // 2):
                ht_pre = gpool.tile([H, chunk], BF16, tag="htp", name="htp")
                nc.vector.tensor_add(ht_pre, h_cur[:, 0:chunk],
                                     h_cur[:, nl // 2 : nl // 2 + chunk])
        h_prev, c_prev = h_cur, c_cur

    # stop level's h/c (2^stop_ll nodes each, bit-reversed order) -> bf16 out
    # (c first: it's ready before h, so its DMA overlaps the final tanh/h)
    ns = 2 ** stop_ll
    nc.sync.dma_start(out=out_hc[:, ns : 2 * ns], in_=c_prev[:, 0:ns])
    nc.sync.dma_start(out=out_hc[:, 0:ns], in_=h_prev[:, 0:ns])


def _emit_body(nc, tc, xt, W, U, b, n_ll, chunk,
               xpool, xtailpool, gpool, ppool, pfpool, hcpool, out_hc, opts={}):

    tanh_pair = opts.get("tanh_pair", 1)

    def _flush_tanh(pend, c_lvl, h_lvl, force):
        if not pend or (len(pend) < tanh_pair and not force):
            return
        a0 = pend[0][0]
        tot = sum(p[1] for p in pend)
        tct = gpool.tile([H, tot], BF16, tag="tanhc", name="tanhc")
        nc.scalar.activation(tct, c_lvl[:, a0 : a0 + tot], TANH)
        off = 0
        for (a, P, go) in pend:
            nc.vector.tensor_mul(h_lvl[:, a : a + P], go, tct[:, off : off + P])
            off += P
        pend.clear()

    def x_chunk(ll, a, P):
        off = 2 ** ll - 1
        pool = xpool if 2 ** ll > chunk else xtailpool
        xt_sb = pool.tile([H, P], BF16, tag="x")
        nc.sync.dma_start(out=xt_sb, in_=xt[:, off + a : off + a + P])
        return xt_sb

    # ---- leaves (ll = n_ll): c = i*u, h = o*tanh(c) ----
    nl = 2 ** n_ll
    h_prev = hcpool.tile([H, nl], BF16, tag=f"h{n_ll}")
    c_prev = hcpool.tile([H, nl], BF16, tag=f"c{n_ll}")
    pend = []
    for a in range(0, nl, chunk):
        P = min(chunk, nl - a)
        xs = x_chunk(n_ll, a, P)
        if opts.get("io_merge"):
            bT, ones = opts["bT"], opts["ones"]
            if opts.get("sig_merge"):
                pio = pfpool.tile([H, 2 * P], F32, tag="psig", name="pio")
            else:
                pio = ppool.tile([H, 2 * P], F32, tag="pio", name="pio")
            nc.tensor.matmul(pio[:, 0:P], W["i"], xs, start=True, stop=False)
            nc.tensor.matmul(pio[:, 0:P], bT[:, 0, :], ones[:, 0:P], start=False, stop=True)
            nc.tensor.matmul(pio[:, P : 2 * P], W["o"], xs, start=True, stop=False)
            nc.tensor.matmul(pio[:, P : 2 * P], bT[:, 1, :], ones[:, 0:P], start=False, stop=True)
            pu = ppool.tile([H, P], F32, tag="pu", name="pu")
            nc.tensor.matmul(pu, W["u"], xs, start=True, stop=True)
            gio = gpool.tile([H, 2 * P], BF16, tag="gio")
            nc.scalar.activation(gio, pio, SIG)
            gi, go = gio[:, 0:P], gio[:, P : 2 * P]
            gu = gpool.tile([H, P], BF16, tag="gu")
            nc.scalar.activation(gu, pu, TANH, bias=b["u"])
        else:
            ps = {}
            for g in ("i", "o", "u"):
                ps[g] = ppool.tile([H, P], F32, tag=f"p{g}", name=f"p{g}")
                nc.tensor.matmul(ps[g], W[g], xs, start=True, stop=True)
            gi = gpool.tile([H, P], BF16, tag="gi")
            nc.scalar.activation(gi, ps["i"], SIG, bias=b["i"])
            go = gpool.tile([H, P], BF16, tag="go")
            nc.scalar.activation(go, ps["o"], SIG, bias=b["o"])
            gu = gpool.tile([H, P], BF16, tag="gu")
            nc.scalar.activation(gu, ps["u"], TANH, bias=b["u"])
        nc.vector.tensor_mul(c_prev[:, a : a + P], gi, gu)
        pend.append((a, P, go))
        _flush_tanh(pend, c_prev, h_prev, a + P >= nl)

    # ---- internal levels ll = n_ll-1 .. 1 ----
    stop_ll = opts.get("stop_ll", 1)
    for ll in range(n_ll - 1, stop_ll - 1, -1):
        nl = 2 ** ll
        h_cur = hcpool.tile([H, nl], BF16, tag=f"h{ll}")
        c_cur = hcpool.tile([H, nl], BF16, tag=f"c{ll}")
        pend = []
        for a in range(0, nl, chunk):
            P = min(chunk, nl - a)
            xs = x_chunk(ll, a, P)
            # children of parents [a, a+P) sit at the same offsets in
            # the two halves of the (bit-reversed) child level
            h0 = h_prev[:, a : a + P]
            h1 = h_prev[:, nl + a : nl + a + P]
            c0 = c_prev[:, a : a + P]
            c1 = c_prev[:, nl + a : nl + a + P]
            pe_cs = ll <= opts.get("pe_childsum_ll", 0)
            if not pe_cs:
                ht = gpool.tile([H, P], BF16, tag="ht")
                nc.vector.tensor_add(ht, h0, h1)
            if opts.get("sig_merge"):
                bT, ones = opts["bT"], opts["ones"]
                psig = pfpool.tile([H, 4 * P], F32, tag="psig", name="psig")
                for j, g in enumerate(("i", "o")):
                    sl = psig[:, j * P : (j + 1) * P]
                    nc.tensor.matmul(sl, U[g], ht, start=True, stop=False)
                    nc.tensor.matmul(sl, W[g], xs, start=False, stop=False)
                    nc.tensor.matmul(sl, bT[:, j, :], ones[:, 0:P], start=False, stop=True)
                for j, hj in ((2, h0), (3, h1)):
                    sl = psig[:, j * P : (j + 1) * P]
                    nc.tensor.matmul(sl, U["f"], hj, start=True, stop=False)
                    nc.tensor.matmul(sl, W["f"], xs, start=False, stop=False)
                    nc.tensor.matmul(sl, bT[:, 2, :], ones[:, 0:P], start=False, stop=True)
                pu = ppool.tile([H, P], F32, tag="pu", name="pu")
                nc.tensor.matmul(pu, U["u"], ht, start=True, stop=False)
                nc.tensor.matmul(pu, W["u"], xs, start=False, stop=True)
                gs = gpool.tile([H, 4 * P], BF16, tag="gs")
                nc.scalar.activation(gs, psig, SIG)
                gi, go = gs[:, 0:P], gs[:, P : 2 * P]
                gf = gs[:, 2 * P : 4 * P]
                gu = gpool.tile([H, P], BF16, tag="gu")
                nc.scalar.activation(gu, pu, TANH, bias=b["u"])
                iu = gpool.tile([H, P], BF16, tag="iu")
                nc.vector.tensor_mul(iu, gi, gu)
                t0 = gpool.tile([H, P], BF16, tag="t0")
                nc.vector.tensor_mul(t0, gf[:, 0:P], c0)
                t1 = gpool.tile([H, P], BF16, tag="t1")
                nc.vector.tensor_mul(t1, gf[:, P : 2 * P], c1)
                ts = gpool.tile([H, P], BF16, tag="ts")
                nc.vector.tensor_add(ts, t0, t1)
                nc.vector.tensor_add(c_cur[:, a : a + P], iu, ts)
                pend.append((a, P, go))
                _flush_tanh(pend, c_cur, h_cur, a + P >= nl)
                continue
            if opts.get("io_merge"):
                bT, ones = opts["bT"], opts["ones"]
                pio = ppool.tile([H, 2 * P], F32, tag="pio", name="pio")
                for j, g in enumerate(("i", "o")):
                    sl = pio[:, j * P : (j + 1) * P]
                    if pe_cs:
                        nc.tensor.matmul(sl, U[g], h0, start=True, stop=False)
                        nc.tensor.matmul(sl, U[g], h1, start=False, stop=False)
                    else:
                        nc.tensor.matmul(sl, U[g], ht, start=True, stop=False)
                    nc.tensor.matmul(sl, W[g], xs, start=False, stop=False)
                    nc.tensor.matmul(sl, bT[:, j, :], ones[:, 0:P], start=False, stop=True)
                pu = ppool.tile([H, P], F32, tag="pu", name="pu")
                if pe_cs:
                    nc.tensor.matmul(pu, U["u"], h0, start=True, stop=False)
                    nc.tensor.matmul(pu, U["u"], h1, start=False, stop=False)
                else:
                    nc.tensor.matmul(pu, U["u"], ht, start=True, stop=False)
                nc.tensor.matmul(pu, W["u"], xs, start=False, stop=True)
                ps = None
            else:
                ps = {}
                for g in ("i", "o", "u"):
                    ps[g] = ppool.tile([H, P], F32, tag=f"p{g}", name=f"p{g}")
                    nc.tensor.matmul(ps[g], U[g], ht, start=True, stop=False)
                    nc.tensor.matmul(ps[g], W[g], xs, start=False, stop=True)
            if opts.get("f_split"):
                psf0 = pfpool.tile([H, P], F32, tag="pf0", name="psf0")
                psf1 = pfpool.tile([H, P], F32, tag="pf1", name="psf1")
                f_parts = (psf0, psf1)
                nc.tensor.matmul(psf0, U["f"], h0, start=True, stop=False)
                nc.tensor.matmul(psf0, W["f"], xs, start=False, stop=True)
                nc.tensor.matmul(psf1, U["f"], h1, start=True, stop=False)
                nc.tensor.matmul(psf1, W["f"], xs, start=False, stop=True)
            else:
                psf = pfpool.tile([H, 2 * P], F32, tag="pf")
                f_parts = None
                nc.tensor.matmul(psf[:, 0:P], U["f"], h0, start=True, stop=False)
                nc.tensor.matmul(psf[:, 0:P], W["f"], xs, start=False, stop=True)
                nc.tensor.matmul(psf[:, P : 2 * P], U["f"], h1, start=True, stop=False)
                nc.tensor.matmul(psf[:, P : 2 * P], W["f"], xs, start=False, stop=True)
            if ps is None:
                gio = gpool.tile([H, 2 * P], BF16, tag="gio")
                nc.scalar.activation(gio, pio, SIG)
                gi, go = gio[:, 0:P], gio[:, P : 2 * P]
                gu = gpool.tile([H, P], BF16, tag="gu")
                nc.scalar.activation(gu, pu, TANH, bias=b["u"])
            else:
                gi = gpool.tile([H, P], BF16, tag="gi")
                nc.scalar.activation(gi, ps["i"], SIG, bias=b["i"])
                go = gpool.tile([H, P], BF16, tag="go")
                nc.scalar.activation(go, ps["o"], SIG, bias=b["o"])
                gu = gpool.tile([H, P], BF16, tag="gu")
                nc.scalar.activation(gu, ps["u"], TANH, bias=b["u"])
            gf = gpool.tile([H, 2 * P], BF16, tag="gf")
            if f_parts is not None:
                nc.scalar.activation(gf[:, 0:P], f_parts[0], SIG, bias=b["f"])
                nc.scalar.activation(gf[:, P : 2 * P], f_parts[1], SIG, bias=b["f"])
            else:
                nc.scalar.activation(gf, psf, SIG, bias=b["f"])
            iu = gpool.tile([H, P], BF16, tag="iu")
            nc.vector.tensor_mul(iu, gi, gu)
            t0 = gpool.tile([H, P], BF16, tag="t0")
            nc.vector.tensor_mul(t0, gf[:, 0:P], c0)
            t1 = gpool.tile([H, P], BF16, tag="t1")
            nc.vector.tensor_mul(t1, gf[:, P : 2 * P], c1)
            ts = gpool.tile([H, P], BF16, tag="ts")
            nc.vector.tensor_add(ts, t0, t1)
            nc.vector.tensor_add(c_cur[:, a : a + P], iu, ts)
            pend.append((a, P, go))
            _flush_tanh(pend, c_cur, h_cur, a + P >= nl)
        h_prev, c_prev = h_cur, c_cur

    # ll=1 h/c (2 nodes) -> fp32 output [H, 4] = [h0 h1 c0 c1]
    res = gpool.tile([H, 4], F32, tag="res")
    nc.vector.tensor_copy(res[:, 0:2], h_prev[:, 0:2])
    nc.vector.tensor_copy(res[:, 2:4], c_prev[:, 0:2])
    nc.sync.dma_start(out=out_hc[:, :], in_=res)



def _build_program(n_ll=N_LL, chunk=CHUNK, dyn_loop=False, **opts):
    nc = bacc.Bacc("TRN2", target_bir_lowering=False, debug=False)
    nsub = 2 ** (n_ll + 1) - 1

    xpack = opts.get("xpack")
    xoff = 8 * H if xpack else 0
    xt = nc.declare_dram_parameter("xt", [H, xoff + nsub], BF16, isOutput=False)
    niter_dram = None
    if dyn_loop:
        niter_dram = nc.declare_dram_parameter("niter", [1, 1], mybir.dt.uint32, isOutput=False)
    bT_dram = None
    if opts.get("io_merge") or opts.get("io2"):
        bT_dram = nc.declare_dram_parameter("bT_all", [1, 3, H], BF16, isOutput=False)
    # wu_all[:, j, :]: W_i W_f W_o W_u U_i U_f U_o U_u (j = 0..7)
    wu_dram = None
    if not xpack:
        wu_dram = nc.declare_dram_parameter("wu_all", [H, 8, H], BF16, isOutput=False)
    b_dram = nc.declare_dram_parameter("b_all", [H, 4], F32, isOutput=False)
    n_stop = 2 ** opts.get("stop_ll", 1)
    out_hc = nc.declare_dram_parameter("out_hc", [H, 2 * n_stop], BF16, isOutput=True)

    with tile.TileContext(nc) as tc:
        with (
            tc.tile_pool(name="wpool", bufs=1) as wpool,
            tc.tile_pool(name="hc", bufs=1) as hcpool,
            tc.tile_pool(name="xs", bufs=opts.get("xpool_bufs", 4)) as xpool,
            tc.tile_pool(name="xtail", bufs=3) as xtailpool,
            tc.tile_pool(name="gates", bufs=opts.get("gpool_bufs", 3)) as gpool,
            tc.tile_pool(name="ps", bufs=opts.get("iou_bufs", 2), space=bass.MemorySpace.PSUM) as ppool,
            tc.tile_pool(name="psf", bufs=opts.get("pf_bufs", 1), space=bass.MemorySpace.PSUM) as pfpool,
        ):
            x0_sb = None
            b_sb = wpool.tile([H, 4], F32, tag="b", name="b_sb")
            if xpack:
                # weights ride as the first 8*H columns of xt: ONE DMA lands
                # W, U and the first leaf x chunk (fewer HWDGE slots + DMA
                # completion semaphores on the startup critical path).
                x0w = min(chunk, 2 ** n_ll)
                wx_sb = wpool.tile([H, xoff + x0w], BF16, tag="wu", name="wx_sb")
                nc.sync.dma_start(out=wx_sb, in_=xt[:, 0 : xoff + x0w])
                nc.sync.dma_start(out=b_sb, in_=b_dram[:, :])
                W = {g: wx_sb[:, j * H : (j + 1) * H] for j, g in enumerate(GATES)}
                U = {g: wx_sb[:, (4 + j) * H : (5 + j) * H] for j, g in enumerate(GATES)}
                x0_sb = wx_sb[:, xoff : xoff + x0w]
            else:
                wu_sb = wpool.tile([H, 8, H], BF16, tag="wu", name="wu_sb")
                if opts.get("x0pf", True):
                    # land b + W_i first, then the first leaf x chunk, then
                    # the remaining weights — the first sigmoid only needs
                    # W_i/x0/b_i.
                    nc.sync.dma_start(out=b_sb, in_=b_dram[:, :])
                    nc.sync.dma_start(out=wu_sb[:, 0, :], in_=wu_dram[:, 0, :])
                    x0_sb = xpool.tile([H, min(chunk, 2 ** n_ll)], BF16, tag="x",
                                       name="x0_sb")
                    nc.sync.dma_start(out=x0_sb, in_=xt[:, 2 ** n_ll - 1 :
                                                       2 ** n_ll - 1 + x0_sb.shape[-1]])
                    nc.sync.dma_start(out=wu_sb[:, 1:8, :], in_=wu_dram[:, 1:8, :])
                else:
                    nc.sync.dma_start(out=wu_sb, in_=wu_dram[:, :, :])
                    nc.sync.dma_start(out=b_sb, in_=b_dram[:, :])
                W = {g: wu_sb[:, j, :] for j, g in enumerate(GATES)}
                U = {g: wu_sb[:, 4 + j, :] for j, g in enumerate(GATES)}
            b = {g: b_sb[:, j : j + 1] for j, g in enumerate(GATES)}
            opts = dict(opts, x0=x0_sb, xoff=xoff)
            if opts.get("lsplit"):
                k1t_sb = wpool.tile([H, chunk], BF16, tag="k1t", name="k1t_sb")
                nc.vector.memset(k1t_sb, -0.30798793)
                opts = dict(opts, k1t=k1t_sb)
                if opts.get("ttpoly"):
                    k0t_sb = wpool.tile([H, chunk], BF16, tag="k0t", name="k0t_sb")
                    nc.vector.memset(k0t_sb, 0.99716336)
                    k2t_sb = wpool.tile([H, chunk], BF16, tag="k2t", name="k2t_sb")
                    nc.vector.memset(k2t_sb, 0.0728065)
                    opts = dict(opts, k0t=k0t_sb, k2t=k2t_sb)
            if opts.get("io_merge") or opts.get("io2"):
                bT_sb = wpool.tile([1, 3, H], BF16, tag="bT", name="bT_sb")
                nc.sync.dma_start(out=bT_sb, in_=bT_dram[:, :, :])
                ones_sb = wpool.tile([1, chunk], BF16, tag="ones", name="ones_sb")
                nc.vector.memset(ones_sb, 1.0)
                opts = dict(opts, bT=bT_sb, ones=ones_sb)

            import contextlib
            loop_cm = contextlib.nullcontext()
            if dyn_loop:
                nit_sb = wpool.tile([1, 1], mybir.dt.uint32, tag="nit", name="nit_sb")
                nc.sync.dma_start(out=nit_sb, in_=niter_dram[:, :])
                nit = nc.values_load(nit_sb, min_val=1, max_val=100000,
                                     skip_runtime_bounds_check=True)
                loop_cm = tc.For_i(0, nit, 1)
            with loop_cm:
                emit = _emit_body_wide if opts.get("wide") else _emit_body
                emit(nc, tc, xt, W, U, b, n_ll, chunk,
                     xpool, xtailpool, gpool, ppool, pfpool, hcpool, out_hc,
                     opts)


    nc.finalize()
    return nc


_PROGRAM_CACHE = {}


def _get_program(n_ll=N_LL, chunk=CHUNK, dyn_loop=False):
    key = (n_ll, chunk, dyn_loop)
    if key not in _PROGRAM_CACHE:
        _PROGRAM_CACHE[key] = _build_program(n_ll, chunk, dyn_loop, **BUILD_OPTS)
    return _PROGRAM_CACHE[key]


def _bitrev(n_bits):
    """indices 0..2^n-1 in bit-reversed order (as an int array)."""
    n = 2 ** n_bits
    r = np.zeros(n, dtype=np.int64)
    idx = np.arange(n)
    for i in range(n_bits):
        r = (r << 1) | ((idx >> i) & 1)
    return r


def _subtree_index(core, n_ll=N_LL, split=SPLIT, desc=False):
    """Global x-row indices for core's x^T columns (level-major, bit-rev).
    desc=True lists levels leaf-first (the device's read order, for xpack)."""
    parts = []
    lls = range(n_ll, -1, -1) if desc else range(n_ll + 1)
    for ll in lls:
        gl = ll + split
        q = _bitrev(ll)
        parts.append((2 ** gl - 1) + core * (2 ** ll) + q)
    return np.concatenate(parts)


def _prepare(inputs):
    """Host prep: per-core feature-major bf16 x^T (bit-reversed levels) + weights."""
    x = np.asarray(inputs["x"], dtype=np.float32)
    depth = int(inputs["depth"])
    assert depth == DEPTH and x.shape == (2 ** (DEPTH + 1) - 1, H)
    Wf32 = {g: np.asarray(inputs[f"W_{g}"], dtype=np.float32) for g in GATES}
    Uf32 = {g: np.asarray(inputs[f"U_{g}"], dtype=np.float32) for g in GATES}
    bf32 = {g: np.asarray(inputs[f"b_{g}"], dtype=np.float32) for g in GATES}

    x_bf = x.astype(ml_dtypes.bfloat16)
    wu = np.stack([Wf32[g] for g in GATES] + [Uf32[g] for g in GATES], axis=1)
    wu_bf = np.ascontiguousarray(wu.astype(ml_dtypes.bfloat16))
    xpack = BUILD_OPTS.get("xpack")
    shared = {
        "b_all": np.ascontiguousarray(np.stack([bf32[g] for g in GATES], axis=1)),
    }
    if not xpack:
        shared["wu_all"] = wu_bf
    if BUILD_OPTS.get("io_merge") or BUILD_OPTS.get("io2"):
        bT = np.stack([bf32["i"], bf32["o"], bf32["f"]], axis=0)[None]  # [1,3,H]
        shared["bT_all"] = np.ascontiguousarray(bT.astype(ml_dtypes.bfloat16))
    wu_cols = wu_bf.reshape(H, 8 * H)       # [H, 1024]: W/U as xt head columns
    in_maps = []
    for k in range(NCORES):
        idx = _subtree_index(k, desc=bool(xpack))
        m = dict(shared)
        xt = x_bf[idx].T
        if xpack:
            xt = np.concatenate([wu_cols, xt], axis=1)
        m["xt"] = np.ascontiguousarray(xt)
        in_maps.append(m)
    return in_maps, x, Wf32, Uf32, bf32


def _merge_top(results, x, Wf32, Uf32, bf32):
    """Host: combine per-core level-STOP_LL h/c (bit-reversed node order) and
    run the remaining global levels (split+stop_ll-1 .. 0) in fp32."""
    stop_ll = BUILD_OPTS.get("stop_ll", 1)
    ns = 2 ** stop_ll                       # nodes per core at the stop level
    top_gl = SPLIT + stop_ll                # global level of the stop level
    nt = 2 ** top_gl                        # total nodes at that global level
    q = _bitrev(stop_ll)
    h = np.empty((nt, H), dtype=np.float32)
    c = np.empty((nt, H), dtype=np.float32)
    for k in range(NCORES):
        r = np.asarray(results[k]["out_hc"], dtype=np.float32)
        h[k * ns + q] = r[:, 0:ns].T
        c[k * ns + q] = r[:, ns : 2 * ns].T

    def sigmoid(v):
        return 1.0 / (1.0 + np.exp(-v))

    ntop = 2 ** top_gl - 1                  # internal nodes above the stop level
    Xg = {g: x[:ntop] @ Wf32[g] + bf32[g] for g in GATES}
    for level in range(top_gl - 1, -1, -1):
        s0, nl = 2 ** level - 1, 2 ** level
        ch = h.reshape(nl, 2, H)
        cc = c.reshape(nl, 2, H)
        ht = ch.sum(axis=1)
        i = sigmoid(Xg["i"][s0 : s0 + nl] + ht @ Uf32["i"])
        o = sigmoid(Xg["o"][s0 : s0 + nl] + ht @ Uf32["o"])
        u = np.tanh(Xg["u"][s0 : s0 + nl] + ht @ Uf32["u"])
        f = sigmoid(Xg["f"][s0 : s0 + nl][:, None, :] + ch @ Uf32["f"])
        c = i * u + (f * cc).sum(axis=1)
        h = o * np.tanh(c)

    return np.stack([h[0], c[0]]).astype(np.float32)


def kernel(**inputs):
    in_maps, x, Wf32, Uf32, bf32 = _prepare(inputs)
    nc = _get_program()
    res = run_bass_kernel_spmd(nc, in_maps, core_ids=list(range(NCORES)), trace=TRACE)
    globals()["LAST"] = res
    return _merge_top(res.results, x, Wf32, Uf32, bf32)

